# revision 11
# baseline (speedup 1.0000x reference)
"""Trainium2 Bass kernel for MinGRU (nn_MinGRU_53420803228268).

Math: the reference computes the MinGRU recurrence in log space (Heinsen
scan). Direct space is algebraically identical and numerically benign here
(coefficients in (0,1), values > 0; validated to ~3e-7 rel err vs an fp64
log-space oracle):

    hg   = X @ W_hg.T                  # [B,S,2Di]; h_tilde = hg[:Di], gate = hg[Di:]
    z    = sigmoid(gate);  a = sigmoid(-gate) = 1-z
    g(x) = relu(x) + min(sigmoid(x), 0.5)        # == where(x>=0, x+0.5, sigmoid(x))
    b    = z * g(h_tilde)
    h_t  = a_t * h_{t-1} + b_t                   # affine scan along S
    out  = h @ W_out.T ;  next_hidden = h[:, -1]

Sharding: 8 cores = 4 batches x 2 halves of Di (512 channels each). The
scan stays on-core (DVE tensor_tensor_scan, channels on partitions, time on
the free axis). Each core computes a partial out (its 512-channel slice of
the second matmul's contraction); the host sums the two partials per batch.

Layout: host pre-transposes X and the weight slices so both matmul operands
arrive d-major (contraction on partitions) with fully contiguous DMA lines.
Matmuls run as float32r (fp32 data, 1 cycle/row PE mode).
"""

from contextlib import ExitStack

import numpy as np

import concourse.bass as bass
import concourse.mybir as mybir
import concourse.tile as tile
from concourse import bacc
from concourse.bass_utils import run_bass_kernel_spmd

B, S, D, Di = 4, 8192, 512, 1024
NCORES = 8
HALF = Di // 2        # channels per core
CT = HALF // 128      # channel tiles (4)
KT = D // 128         # contraction tiles for matmul1 (4)
CHUNK = 512           # sequence positions per on-chip chunk
NCHUNK = S // CHUNK
ST = CHUNK // 128     # sequence subtiles for matmul2 (4)

F32 = mybir.dt.float32

_prog_cache: dict = {}


def _emit_matmul2(nc, pp, op, outp, wot_sb, h_tiles, s0):
    F32 = mybir.dt.float32
    for st in range(ST):
        po = pp.tile([128, D], F32, tag="po", bufs=2)
        for ct in range(CT):
            nc.tensor.matmul(
                po,
                lhsT=h_tiles[ct][:, st * 128 : (st + 1) * 128],
                rhs=wot_sb[:, ct, :],
                start=(ct == 0),
                stop=(ct == CT - 1),
            )
        o_sb = op.tile([128, D], F32, tag="o")
        nc.scalar.copy(o_sb, po)
        nc.sync.dma_start(out=outp[s0 + st * 128 : s0 + (st + 1) * 128, :], in_=o_sb)


def _build_program(mm_dtype=mybir.dt.float32r):
    nc = bacc.Bacc("TRN2", target_bir_lowering=False)
    xt = nc.dram_tensor("xt", [D, S], mm_dtype, kind="ExternalInput").ap()
    w1t = nc.dram_tensor("w1t", [D, 2 * HALF], mm_dtype, kind="ExternalInput").ap()
    wot = nc.dram_tensor("wot", [HALF, D], mm_dtype, kind="ExternalInput").ap()
    h0 = nc.dram_tensor("h0", [128, CT], F32, kind="ExternalInput").ap()
    outp = nc.dram_tensor("outp", [S, D], F32, kind="ExternalOutput").ap()
    hlast = nc.dram_tensor("hlast", [128, CT], F32, kind="ExternalOutput").ap()

    Sigmoid = mybir.ActivationFunctionType.Sigmoid
    Relu = mybir.ActivationFunctionType.Relu
    alu = mybir.AluOpType

    with ExitStack() as ctx:
        tc = ctx.enter_context(tile.TileContext(nc))
        consts = ctx.enter_context(tc.tile_pool(name="consts", bufs=1))
        xpool = ctx.enter_context(tc.tile_pool(name="xp", bufs=4))
        pp = ctx.enter_context(tc.tile_pool(name="pp", bufs=2, space="PSUM"))
        ew = ctx.enter_context(tc.tile_pool(name="ew", bufs=3))
        hp = ctx.enter_context(tc.tile_pool(name="hp", bufs=3))
        op = ctx.enter_context(tc.tile_pool(name="op", bufs=4))

        # Constants: weights in d-major / i-major layout, initial hidden.
        w1t_sb = consts.tile([128, KT, 2 * HALF], mm_dtype)
        w1t_v = w1t.rearrange("(kt p) e -> p kt e", p=128)
        for eslice in range(8):
            nc.sync.dma_start(
                out=w1t_sb[:, :, eslice * 128 : (eslice + 1) * 128],
                in_=w1t_v[:, :, eslice * 128 : (eslice + 1) * 128],
            )
        wot_sb = consts.tile([128, CT, D], mm_dtype)
        nc.sync.dma_start(out=wot_sb, in_=wot.rearrange("(ct p) d -> p ct d", p=128))
        h0_sb = consts.tile([128, CT], F32)
        nc.sync.dma_start(out=h0_sb, in_=h0)
        hlast_sb = consts.tile([128, CT], F32)

        xt_v = xt.rearrange("(kt p) s -> p kt s", p=128)
        prev_h: list = [None] * CT

        for j in range(NCHUNK):
            s0 = j * CHUNK
            x_sb = xpool.tile([128, KT, CHUNK], mm_dtype, tag="x")
            nc.sync.dma_start(out=x_sb, in_=xt_v[:, :, s0 : s0 + CHUNK])

            h_tiles = []
            for ct in range(CT):
                ph = pp.tile([128, CHUNK], F32, tag="ph", bufs=3)
                pg = pp.tile([128, CHUNK], F32, tag="pg", bufs=3)
                for kt in range(KT):
                    nc.tensor.matmul(
                        ph,
                        lhsT=w1t_sb[:, kt, ct * 128 : (ct + 1) * 128],
                        rhs=x_sb[:, kt, :],
                        start=(kt == 0),
                        stop=(kt == KT - 1),
                    )
                for kt in range(KT):
                    nc.tensor.matmul(
                        pg,
                        lhsT=w1t_sb[
                            :, kt, HALF + ct * 128 : HALF + (ct + 1) * 128
                        ],
                        rhs=x_sb[:, kt, :],
                        start=(kt == 0),
                        stop=(kt == KT - 1),
                    )

                sh = ew.tile([128, CHUNK], F32, tag="sh")
                nc.scalar.activation(sh, ph, Sigmoid)
                z = ew.tile([128, CHUNK], F32, tag="z")
                nc.scalar.activation(z, pg, Sigmoid)
                a = ew.tile([128, CHUNK], F32, tag="a")
                nc.gpsimd.tensor_scalar(
                    out=a, in0=z, scalar1=-1.0, scalar2=1.0, op0=alu.mult, op1=alu.add
                )

                # g(x) = where(x>=0, x+0.5, sigmoid(x)) == max(sigmoid(x), x+0.5)
                # exactly (sigmoid slope <= 1/4), fused into one DVE op.
                g = ew.tile([128, CHUNK], F32, tag="g")
                nc.vector.scalar_tensor_tensor(
                    out=g, in0=ph, scalar=0.5, in1=sh, op0=alu.add, op1=alu.max
                )
                b = ew.tile([128, CHUNK], F32, tag="b")
                nc.gpsimd.tensor_tensor(out=b, in0=z, in1=g, op=alu.mult)

                h = hp.tile([128, CHUNK], mm_dtype, tag=f"h{ct}")
                init = h0_sb[:, ct : ct + 1] if j == 0 else prev_h[ct][:, -1:]
                nc.vector.tensor_tensor_scan(
                    out=h, data0=a, data1=b, initial=init, op0=alu.mult, op1=alu.add
                )
                prev_h[ct] = h
                h_tiles.append(h)

            if j > 0:
                _emit_matmul2(nc, pp, op, outp, wot_sb, pending, (j - 1) * CHUNK)
            pending = h_tiles

        _emit_matmul2(nc, pp, op, outp, wot_sb, pending, (NCHUNK - 1) * CHUNK)

        for ct in range(CT):
            nc.vector.tensor_copy(out=hlast_sb[:, ct : ct + 1], in_=prev_h[ct][:, -1:])
        nc.sync.dma_start(out=hlast, in_=hlast_sb)

    nc.compile()
    return nc


def get_program():
    if "nc" not in _prog_cache:
        _prog_cache["nc"] = _build_program()
    return _prog_cache["nc"]


def make_in_maps(X, hidden, W_hg, W_out):
    X = np.ascontiguousarray(np.asarray(X), dtype=np.float32)
    hidden = np.asarray(hidden, dtype=np.float32)
    W_hg = np.asarray(W_hg, dtype=np.float32)
    W_out = np.asarray(W_out, dtype=np.float32)
    in_maps = []
    for c in range(NCORES):
        b, half = divmod(c, 2)
        sl = slice(half * HALF, (half + 1) * HALF)
        w1 = np.concatenate([W_hg[:Di][sl], W_hg[Di:][sl]], axis=0)  # [2*HALF, D]
        in_maps.append(
            {
                "xt": np.ascontiguousarray(X[b].T),
                "w1t": np.ascontiguousarray(w1.T),
                "wot": np.ascontiguousarray(W_out[:, sl].T),
                "h0": np.ascontiguousarray(hidden[b, 0, sl].reshape(CT, 128).T),
            }
        )
    return in_maps


def combine_results(results):
    out = np.empty((B, S, D), np.float32)
    nh = np.empty((B, 1, Di), np.float32)
    for b in range(B):
        out[b] = results[2 * b]["outp"] + results[2 * b + 1]["outp"]
        for half in range(2):
            hl = results[2 * b + half]["hlast"]  # [128, CT]
            nh[b, 0, half * HALF : (half + 1) * HALF] = hl.T.reshape(HALF)
    return out, nh


def run(X, hidden, W_hg, W_out, trace=False):
    nc = get_program()
    in_maps = make_in_maps(X, hidden, W_hg, W_out)
    res = run_bass_kernel_spmd(nc, in_maps, list(range(NCORES)), trace=trace)
    out, nh = combine_results(res.results)
    return out, nh, res


def kernel(X, hidden, W_hg, W_out):
    out, nh, _ = run(X, hidden, W_hg, W_out)
    return out, nh


# revision 12
# speedup vs baseline: 1.0444x; 1.0444x over previous
"""Trainium2 Bass kernel for MinGRU (nn_MinGRU_53420803228268).

Math: the reference computes the MinGRU recurrence in log space (Heinsen
scan). Direct space is algebraically identical and numerically benign here
(coefficients in (0,1), values > 0; validated to ~3e-7 rel err vs an fp64
log-space oracle):

    hg   = X @ W_hg.T                  # [B,S,2Di]; h_tilde = hg[:Di], gate = hg[Di:]
    z    = sigmoid(gate);  a = sigmoid(-gate) = 1-z
    g(x) = relu(x) + min(sigmoid(x), 0.5)        # == where(x>=0, x+0.5, sigmoid(x))
    b    = z * g(h_tilde)
    h_t  = a_t * h_{t-1} + b_t                   # affine scan along S
    out  = h @ W_out.T ;  next_hidden = h[:, -1]

Sharding: 8 cores = 4 batches x 2 halves of Di (512 channels each). The
scan stays on-core (DVE tensor_tensor_scan, channels on partitions, time on
the free axis). Each core computes a partial out (its 512-channel slice of
the second matmul's contraction); the host sums the two partials per batch.

Layout: host pre-transposes X and the weight slices so both matmul operands
arrive d-major (contraction on partitions) with fully contiguous DMA lines.
Matmuls run as float32r (fp32 data, 1 cycle/row PE mode).
"""

from contextlib import ExitStack

import numpy as np

import concourse.bass as bass
import concourse.mybir as mybir
import concourse.tile as tile
from concourse import bacc
from concourse.bass_utils import run_bass_kernel_spmd

B, S, D, Di = 4, 8192, 512, 1024
NCORES = 8
HALF = Di // 2        # channels per core
CT = HALF // 128      # channel tiles (4)
KT = D // 128         # contraction tiles for matmul1 (4)
CHUNK = 512           # sequence positions per on-chip chunk
NCHUNK = S // CHUNK
ST = CHUNK // 128     # sequence subtiles for matmul2 (4)

F32 = mybir.dt.float32

_prog_cache: dict = {}


def _emit_matmul2(nc, pp, op, outp, wot_sb, h_tiles, s0):
    F32 = mybir.dt.float32
    for st in range(ST):
        po = pp.tile([128, D], F32, tag="po", bufs=2)
        for ct in range(CT):
            nc.tensor.matmul(
                po,
                lhsT=h_tiles[ct][:, st * 128 : (st + 1) * 128],
                rhs=wot_sb[:, ct, :],
                start=(ct == 0),
                stop=(ct == CT - 1),
            )
        o_sb = op.tile([128, D], F32, tag="o")
        nc.scalar.copy(o_sb, po)
        nc.sync.dma_start(out=outp[s0 + st * 128 : s0 + (st + 1) * 128, :], in_=o_sb)


def _build_program(mm_dtype=mybir.dt.float32r):
    nc = bacc.Bacc("TRN2", target_bir_lowering=False)
    xt = nc.dram_tensor("xt", [D, S], mm_dtype, kind="ExternalInput").ap()
    w1t = nc.dram_tensor("w1t", [D, 2 * HALF], mm_dtype, kind="ExternalInput").ap()
    wot = nc.dram_tensor("wot", [HALF, D], mm_dtype, kind="ExternalInput").ap()
    h0 = nc.dram_tensor("h0", [128, CT], F32, kind="ExternalInput").ap()
    outp = nc.dram_tensor("outp", [S, D], F32, kind="ExternalOutput").ap()
    hlast = nc.dram_tensor("hlast", [128, CT], F32, kind="ExternalOutput").ap()

    Sigmoid = mybir.ActivationFunctionType.Sigmoid
    Relu = mybir.ActivationFunctionType.Relu
    alu = mybir.AluOpType

    with ExitStack() as ctx:
        tc = ctx.enter_context(tile.TileContext(nc))
        consts = ctx.enter_context(tc.tile_pool(name="consts", bufs=1))
        xpool = ctx.enter_context(tc.tile_pool(name="xp", bufs=4))
        pp = ctx.enter_context(tc.tile_pool(name="pp", bufs=2, space="PSUM"))
        ew = ctx.enter_context(tc.tile_pool(name="ew", bufs=3))
        hp = ctx.enter_context(tc.tile_pool(name="hp", bufs=3))
        op = ctx.enter_context(tc.tile_pool(name="op", bufs=4))

        # Constants: weights in d-major / i-major layout, initial hidden.
        w1t_sb = consts.tile([128, KT, 2 * HALF], mm_dtype)
        w1t_v = w1t.rearrange("(kt p) e -> p kt e", p=128)
        for eslice in range(8):
            nc.sync.dma_start(
                out=w1t_sb[:, :, eslice * 128 : (eslice + 1) * 128],
                in_=w1t_v[:, :, eslice * 128 : (eslice + 1) * 128],
            )
        wot_sb = consts.tile([128, CT, D], mm_dtype)
        nc.sync.dma_start(out=wot_sb, in_=wot.rearrange("(ct p) d -> p ct d", p=128))
        h0_sb = consts.tile([128, CT], F32)
        nc.sync.dma_start(out=h0_sb, in_=h0)
        hlast_sb = consts.tile([128, CT], F32)

        xt_v = xt.rearrange("(kt p) s -> p kt s", p=128)
        prev_h: list = [None] * CT

        for j in range(NCHUNK):
            s0 = j * CHUNK
            x_sb = xpool.tile([128, KT, CHUNK], mm_dtype, tag="x")
            nc.sync.dma_start(out=x_sb, in_=xt_v[:, :, s0 : s0 + CHUNK])

            h_tiles = []
            for ct in range(CT):
                ph = pp.tile([128, CHUNK], F32, tag="ph", bufs=3)
                pg = pp.tile([128, CHUNK], F32, tag="pg", bufs=3)
                for kt in range(KT):
                    nc.tensor.matmul(
                        ph,
                        lhsT=w1t_sb[:, kt, ct * 128 : (ct + 1) * 128],
                        rhs=x_sb[:, kt, :],
                        start=(kt == 0),
                        stop=(kt == KT - 1),
                    )
                for kt in range(KT):
                    nc.tensor.matmul(
                        pg,
                        lhsT=w1t_sb[
                            :, kt, HALF + ct * 128 : HALF + (ct + 1) * 128
                        ],
                        rhs=x_sb[:, kt, :],
                        start=(kt == 0),
                        stop=(kt == KT - 1),
                    )

                sh = ew.tile([128, CHUNK], F32, tag="sh")
                nc.scalar.activation(sh, ph, Sigmoid)
                z = ew.tile([128, CHUNK], F32, tag="z")
                nc.scalar.activation(z, pg, Sigmoid)
                a = ew.tile([128, CHUNK], F32, tag="a")
                nc.gpsimd.tensor_scalar(
                    out=a, in0=z, scalar1=-1.0, scalar2=1.0, op0=alu.mult, op1=alu.add
                )

                # g(x) = where(x>=0, x+0.5, sigmoid(x)) == max(sigmoid(x), x+0.5)
                # exactly (sigmoid slope <= 1/4), fused into one DVE op.
                g = ew.tile([128, CHUNK], F32, tag="g")
                nc.vector.scalar_tensor_tensor(
                    out=g, in0=ph, scalar=0.5, in1=sh, op0=alu.add, op1=alu.max
                )
                b = ew.tile([128, CHUNK], F32, tag="b")
                nc.vector.tensor_tensor(out=b, in0=z, in1=g, op=alu.mult)

                h = hp.tile([128, CHUNK], mm_dtype, tag=f"h{ct}")
                init = h0_sb[:, ct : ct + 1] if j == 0 else prev_h[ct][:, -1:]
                nc.vector.tensor_tensor_scan(
                    out=h, data0=a, data1=b, initial=init, op0=alu.mult, op1=alu.add
                )
                prev_h[ct] = h
                h_tiles.append(h)

            if j > 0:
                _emit_matmul2(nc, pp, op, outp, wot_sb, pending, (j - 1) * CHUNK)
            pending = h_tiles

        _emit_matmul2(nc, pp, op, outp, wot_sb, pending, (NCHUNK - 1) * CHUNK)

        for ct in range(CT):
            nc.vector.tensor_copy(out=hlast_sb[:, ct : ct + 1], in_=prev_h[ct][:, -1:])
        nc.sync.dma_start(out=hlast, in_=hlast_sb)

    nc.compile()
    return nc


def get_program():
    if "nc" not in _prog_cache:
        _prog_cache["nc"] = _build_program()
    return _prog_cache["nc"]


def make_in_maps(X, hidden, W_hg, W_out):
    X = np.ascontiguousarray(np.asarray(X), dtype=np.float32)
    hidden = np.asarray(hidden, dtype=np.float32)
    W_hg = np.asarray(W_hg, dtype=np.float32)
    W_out = np.asarray(W_out, dtype=np.float32)
    in_maps = []
    for c in range(NCORES):
        b, half = divmod(c, 2)
        sl = slice(half * HALF, (half + 1) * HALF)
        w1 = np.concatenate([W_hg[:Di][sl], W_hg[Di:][sl]], axis=0)  # [2*HALF, D]
        in_maps.append(
            {
                "xt": np.ascontiguousarray(X[b].T),
                "w1t": np.ascontiguousarray(w1.T),
                "wot": np.ascontiguousarray(W_out[:, sl].T),
                "h0": np.ascontiguousarray(hidden[b, 0, sl].reshape(CT, 128).T),
            }
        )
    return in_maps


def combine_results(results):
    out = np.empty((B, S, D), np.float32)
    nh = np.empty((B, 1, Di), np.float32)
    for b in range(B):
        out[b] = results[2 * b]["outp"] + results[2 * b + 1]["outp"]
        for half in range(2):
            hl = results[2 * b + half]["hlast"]  # [128, CT]
            nh[b, 0, half * HALF : (half + 1) * HALF] = hl.T.reshape(HALF)
    return out, nh


def run(X, hidden, W_hg, W_out, trace=False):
    nc = get_program()
    in_maps = make_in_maps(X, hidden, W_hg, W_out)
    res = run_bass_kernel_spmd(nc, in_maps, list(range(NCORES)), trace=trace)
    out, nh = combine_results(res.results)
    return out, nh, res


def kernel(X, hidden, W_hg, W_out):
    out, nh, _ = run(X, hidden, W_hg, W_out)
    return out, nh


# revision 13
# speedup vs baseline: 1.0679x; 1.0225x over previous
"""Trainium2 Bass kernel for MinGRU (nn_MinGRU_53420803228268).

Math: the reference computes the MinGRU recurrence in log space (Heinsen
scan). Direct space is algebraically identical and numerically benign here
(coefficients in (0,1), values > 0; validated to ~3e-7 rel err vs an fp64
log-space oracle):

    hg   = X @ W_hg.T                  # [B,S,2Di]; h_tilde = hg[:Di], gate = hg[Di:]
    z    = sigmoid(gate);  a = sigmoid(-gate) = 1-z
    g(x) = relu(x) + min(sigmoid(x), 0.5)        # == where(x>=0, x+0.5, sigmoid(x))
    b    = z * g(h_tilde)
    h_t  = a_t * h_{t-1} + b_t                   # affine scan along S
    out  = h @ W_out.T ;  next_hidden = h[:, -1]

Sharding: 8 cores = 4 batches x 2 halves of Di (512 channels each). The
scan stays on-core (DVE tensor_tensor_scan, channels on partitions, time on
the free axis). Each core computes a partial out (its 512-channel slice of
the second matmul's contraction); the host sums the two partials per batch.

Layout: host pre-transposes X and the weight slices so both matmul operands
arrive d-major (contraction on partitions) with fully contiguous DMA lines.
Matmuls run as float32r (fp32 data, 1 cycle/row PE mode).
"""

from contextlib import ExitStack

import numpy as np

import concourse.bass as bass
import concourse.mybir as mybir
import concourse.tile as tile
from concourse import bacc
from concourse.bass_utils import run_bass_kernel_spmd

B, S, D, Di = 4, 8192, 512, 1024
NCORES = 8
HALF = Di // 2        # channels per core
CT = HALF // 128      # channel tiles (4)
KT = D // 128         # contraction tiles for matmul1 (4)
CHUNK = 512           # sequence positions per on-chip chunk
NCHUNK = S // CHUNK
ST = CHUNK // 128     # sequence subtiles for matmul2 (4)

F32 = mybir.dt.float32

_prog_cache: dict = {}


def _emit_matmul2(nc, pp, op, outp, wot_sb, h_tiles, s0):
    F32 = mybir.dt.float32
    for st in range(ST):
        po = pp.tile([128, D], F32, tag="po", bufs=2)
        for ct in range(CT):
            nc.tensor.matmul(
                po,
                lhsT=h_tiles[ct][:, st * 128 : (st + 1) * 128],
                rhs=wot_sb[:, ct, :],
                start=(ct == 0),
                stop=(ct == CT - 1),
            )
        o_sb = op.tile([128, D], F32, tag="o")
        nc.scalar.copy(o_sb, po)
        nc.sync.dma_start(out=outp[s0 + st * 128 : s0 + (st + 1) * 128, :], in_=o_sb)


def _build_program(mm_dtype=mybir.dt.float32r):
    nc = bacc.Bacc("TRN2", target_bir_lowering=False)
    xt = nc.dram_tensor("xt", [D, S], mm_dtype, kind="ExternalInput").ap()
    w1t = nc.dram_tensor("w1t", [D, 2 * HALF], mm_dtype, kind="ExternalInput").ap()
    wot = nc.dram_tensor("wot", [HALF, D], mm_dtype, kind="ExternalInput").ap()
    h0 = nc.dram_tensor("h0", [128, CT], F32, kind="ExternalInput").ap()
    outp = nc.dram_tensor("outp", [S, D], F32, kind="ExternalOutput").ap()
    hlast = nc.dram_tensor("hlast", [128, CT], F32, kind="ExternalOutput").ap()

    Sigmoid = mybir.ActivationFunctionType.Sigmoid
    Relu = mybir.ActivationFunctionType.Relu
    alu = mybir.AluOpType

    with ExitStack() as ctx:
        tc = ctx.enter_context(tile.TileContext(nc))
        consts = ctx.enter_context(tc.tile_pool(name="consts", bufs=1))
        xpool = ctx.enter_context(tc.tile_pool(name="xp", bufs=4))
        pp = ctx.enter_context(tc.tile_pool(name="pp", bufs=2, space="PSUM"))
        ew = ctx.enter_context(tc.tile_pool(name="ew", bufs=3))
        hp = ctx.enter_context(tc.tile_pool(name="hp", bufs=3))
        op = ctx.enter_context(tc.tile_pool(name="op", bufs=4))

        # Constants: weights in d-major / i-major layout, initial hidden.
        w1t_sb = consts.tile([128, KT, 2 * HALF], mm_dtype)
        w1t_v = w1t.rearrange("(kt p) e -> p kt e", p=128)
        for eslice in range(8):
            nc.sync.dma_start(
                out=w1t_sb[:, :, eslice * 128 : (eslice + 1) * 128],
                in_=w1t_v[:, :, eslice * 128 : (eslice + 1) * 128],
            )
        wot_sb = consts.tile([128, CT, D], mm_dtype)
        nc.sync.dma_start(out=wot_sb, in_=wot.rearrange("(ct p) d -> p ct d", p=128))
        h0_sb = consts.tile([128, CT], F32)
        nc.sync.dma_start(out=h0_sb, in_=h0)
        hlast_sb = consts.tile([128, CT], F32)

        xt_v = xt.rearrange("(kt p) s -> p kt s", p=128)
        prev_h: list = [None] * CT

        for j in range(NCHUNK):
            s0 = j * CHUNK
            x_sb = xpool.tile([128, KT, CHUNK], mm_dtype, tag="x")
            if j == 0:
                # split the first chunk's load per k-tile so the first matmul
                # only waits for the slice it reads
                for kt in range(KT):
                    nc.sync.dma_start(
                        out=x_sb[:, kt, :], in_=xt_v[:, kt, s0 : s0 + CHUNK]
                    )
            else:
                nc.sync.dma_start(out=x_sb, in_=xt_v[:, :, s0 : s0 + CHUNK])

            h_tiles = []
            for ct in range(CT):
                ph = pp.tile([128, CHUNK], F32, tag="ph", bufs=3)
                pg = pp.tile([128, CHUNK], F32, tag="pg", bufs=3)
                for kt in range(KT):
                    nc.tensor.matmul(
                        ph,
                        lhsT=w1t_sb[:, kt, ct * 128 : (ct + 1) * 128],
                        rhs=x_sb[:, kt, :],
                        start=(kt == 0),
                        stop=(kt == KT - 1),
                    )
                for kt in range(KT):
                    nc.tensor.matmul(
                        pg,
                        lhsT=w1t_sb[
                            :, kt, HALF + ct * 128 : HALF + (ct + 1) * 128
                        ],
                        rhs=x_sb[:, kt, :],
                        start=(kt == 0),
                        stop=(kt == KT - 1),
                    )

                sh = ew.tile([128, CHUNK], F32, tag="sh")
                nc.scalar.activation(sh, ph, Sigmoid)
                z = ew.tile([128, CHUNK], F32, tag="z")
                nc.scalar.activation(z, pg, Sigmoid)
                a = ew.tile([128, CHUNK], F32, tag="a")
                nc.gpsimd.tensor_scalar(
                    out=a, in0=z, scalar1=-1.0, scalar2=1.0, op0=alu.mult, op1=alu.add
                )

                # g(x) = where(x>=0, x+0.5, sigmoid(x)) == max(sigmoid(x), x+0.5)
                # exactly (sigmoid slope <= 1/4), fused into one DVE op.
                g = ew.tile([128, CHUNK], F32, tag="g")
                nc.vector.scalar_tensor_tensor(
                    out=g, in0=ph, scalar=0.5, in1=sh, op0=alu.add, op1=alu.max
                )
                b = ew.tile([128, CHUNK], F32, tag="b")
                nc.vector.tensor_tensor(out=b, in0=z, in1=g, op=alu.mult)

                h = hp.tile([128, CHUNK], mm_dtype, tag=f"h{ct}")
                init = h0_sb[:, ct : ct + 1] if j == 0 else prev_h[ct][:, -1:]
                nc.vector.tensor_tensor_scan(
                    out=h, data0=a, data1=b, initial=init, op0=alu.mult, op1=alu.add
                )
                prev_h[ct] = h
                h_tiles.append(h)

            if j > 0:
                _emit_matmul2(nc, pp, op, outp, wot_sb, pending, (j - 1) * CHUNK)
            pending = h_tiles

        _emit_matmul2(nc, pp, op, outp, wot_sb, pending, (NCHUNK - 1) * CHUNK)

        for ct in range(CT):
            nc.vector.tensor_copy(out=hlast_sb[:, ct : ct + 1], in_=prev_h[ct][:, -1:])
        nc.sync.dma_start(out=hlast, in_=hlast_sb)

    nc.compile()
    return nc


def get_program():
    if "nc" not in _prog_cache:
        _prog_cache["nc"] = _build_program()
    return _prog_cache["nc"]


def make_in_maps(X, hidden, W_hg, W_out):
    X = np.ascontiguousarray(np.asarray(X), dtype=np.float32)
    hidden = np.asarray(hidden, dtype=np.float32)
    W_hg = np.asarray(W_hg, dtype=np.float32)
    W_out = np.asarray(W_out, dtype=np.float32)
    in_maps = []
    for c in range(NCORES):
        b, half = divmod(c, 2)
        sl = slice(half * HALF, (half + 1) * HALF)
        w1 = np.concatenate([W_hg[:Di][sl], W_hg[Di:][sl]], axis=0)  # [2*HALF, D]
        in_maps.append(
            {
                "xt": np.ascontiguousarray(X[b].T),
                "w1t": np.ascontiguousarray(w1.T),
                "wot": np.ascontiguousarray(W_out[:, sl].T),
                "h0": np.ascontiguousarray(hidden[b, 0, sl].reshape(CT, 128).T),
            }
        )
    return in_maps


def combine_results(results):
    out = np.empty((B, S, D), np.float32)
    nh = np.empty((B, 1, Di), np.float32)
    for b in range(B):
        out[b] = results[2 * b]["outp"] + results[2 * b + 1]["outp"]
        for half in range(2):
            hl = results[2 * b + half]["hlast"]  # [128, CT]
            nh[b, 0, half * HALF : (half + 1) * HALF] = hl.T.reshape(HALF)
    return out, nh


def run(X, hidden, W_hg, W_out, trace=False):
    nc = get_program()
    in_maps = make_in_maps(X, hidden, W_hg, W_out)
    res = run_bass_kernel_spmd(nc, in_maps, list(range(NCORES)), trace=trace)
    out, nh = combine_results(res.results)
    return out, nh, res


def kernel(X, hidden, W_hg, W_out):
    out, nh, _ = run(X, hidden, W_hg, W_out)
    return out, nh
